# revision 1
# baseline (speedup 1.0000x reference)
"""BandSplit (BSRNN-style) Trainium2 kernel.

Reference computation (per batch sample, per band of width w, ch = 4w):
  h   = moveaxis(x[:, :, s:e, :, :], -1, 1).reshape(B, ch, T)   # channels (r, c, f)
  hn  = (h - mu) * rsqrt(var + eps) * gamma + beta              # GroupNorm(1, ch) over (ch, T)
  y   = W_band @ hn + b_band                                    # [128, T]
  out = stack over bands -> [B, 128, 31, T]

Folded form used here (r_b = rsqrt(var+eps); mu, r_b are per band+sample):
  y = r_b * (Wg @ h) + (v + b_band - r_b*mu*u)
  Wg = W * gamma (per column), u = Wg @ 1, v = W @ beta
so the big matmul runs on RAW h (no normalization pass over the data) and the
normalization is applied as a per-band scalar scale + per-output-channel bias
to the matmul output. Wg/u/v are parameter-only values and are prepared on the
host together with the weight layout packing; everything that touches x (the
matmuls, the mean/variance statistics, normalization, bias) runs on device.

Sharding: data-parallel over batch B=8 across the 8 NeuronCores (sample b on
core b); parameters replicated. Inside a core, h is never materialized:
matmuls read the natively-laid-out staged x tiles (partitions = (c, f) rows,
free = interleaved (t, r)) with a stride-2 free-dim access pattern selecting
the real/imag plane. Per-band sums come from one-hot-stationary matmuls
(partition reduction on the PE); sums of squares from scalar-engine
Square+accumulate passes. x is staged in multi-band "super tiles" so each
DMA moves >= ~1 MB (wide hardware-queue fan-out).
"""

import numpy as np

import concourse.bass as bass
import concourse.tile as tile
from concourse import bacc, mybir

F32 = mybir.dt.float32
F32R = mybir.dt.float32r
AFT = mybir.ActivationFunctionType
ALU = mybir.AluOpType

# ---------------------------------------------------------------- problem dims
WIDTHS = [25] * 10 + [50] * 12 + [100] * 8 + [399]
NBANDS = len(WIDTHS)          # 31
C_IN = 2
T = 512
OUT_CH = 128
EPS = 1e-5
F_TOT = 2049
N_CORES = 8
N_WTP_PIECES = 4
OUT_GROUPS = [(0, 8), (8, 16), (16, 24), (24, 31)]

_STARTS = np.concatenate([[0], np.cumsum(WIDTHS)]).astype(int)
_CHOFF = np.concatenate([[0], np.cumsum([4 * w for w in WIDTHS])]).astype(int)


def _tables():
    """Super-tile staging plan + weight-chunk slots + S2 strip columns.

    Each super tile is one DMA; its free dim indexes "subs". A sub is one
    former staged tile: partitions = (c, f) rows of one band (c-stacked when
    2w <= 128), free row = interleaved (t, r) of 1024 floats. Each sub feeds
    two weight-chunk slots (one per r-plane), each a contiguous channel range
    (channel index within band = r*2w + c*w + f).
    """
    supers = []  # dict: kind, bands/c info for DMA AP, subs: [(band, K, [(a,e),(a,e)])]

    def band_sub(i, w, off):
        # c-stacked sub for a 2w<=128 band
        return (i, 2 * w, [(off, off + 2 * w), (off + 2 * w, off + 4 * w)])

    # class A: w=25 bands 0..9, pairs; band b's rows at partition offset 64
    # (spreads DMA traffic across more engines; the two bands' matmuls run in
    # different PE row-groups concurrently)
    for a in range(5):
        i0 = 2 * a
        supers.append(dict(
            kind="pair64", f0=int(_STARTS[i0]), w=25,
            subs=[band_sub(i0 + j, 25, int(_CHOFF[i0 + j])) for j in range(2)],
            p0=[0, 64],
        ))
    # class B: w=50 bands 10..21, pairs
    for a in range(6):
        i0 = 10 + 2 * a
        supers.append(dict(
            kind="pair", f0=int(_STARTS[i0]), w=50,
            subs=[band_sub(i0 + j, 50, int(_CHOFF[i0 + j])) for j in range(2)],
        ))
    # class C: w=100 bands 22..29, one super per band, subs = (c0, c1)
    for i in range(22, 30):
        off = int(_CHOFF[i])
        w = 100
        supers.append(dict(
            kind="cpair", f0=int(_STARTS[i]), w=w, nf=w,
            subs=[(i, w, [(off + r * 2 * w + c * w, off + r * 2 * w + (c + 1) * w)
                          for r in range(2)]) for c in range(2)],
        ))
    # class D: band 30 (w=399), supers per f-chunk, subs = (c0, c1)
    i = 30
    off = int(_CHOFF[i])
    w = 399
    for f0 in range(0, w, 128):
        f1 = min(f0 + 128, w)
        supers.append(dict(
            kind="cpair", f0=int(_STARTS[i]) + f0, w=w, nf=f1 - f0,
            subs=[(i, f1 - f0,
                   [(off + r * 2 * w + c * w + f0, off + r * 2 * w + c * w + f1)
                    for r in range(2)]) for c in range(2)],
        ))

    # weight slots: one per (super, sub, r), in traversal order
    slots = []
    for si, sup in enumerate(supers):
        p0s = sup.get("p0", [0] * len(sup["subs"]))
        span = max(p0s[j] + sup["subs"][j][1] for j in range(len(sup["subs"])))
        for j, (band, K, chs) in enumerate(sup["subs"]):
            for r in range(2):
                slots.append(dict(super=si, sub=j, r=r, ch=chs[r], p0=p0s[j],
                                  span=span))

    # S2 strip columns: one ACT op per (super, band-different subs) or per super
    # (when both subs are the same band). Per-band ranges padded to EVEN width.
    s2ops = []  # (super_idx, sub_list, band, col)
    band_ncols = [0] * NBANDS
    per_super_ops = []
    for si, sup in enumerate(supers):
        bands = {b for (b, _, _) in sup["subs"]}
        if len(bands) == 1:
            per_super_ops.append((si, list(range(len(sup["subs"]))), sup["subs"][0][0]))
        else:
            for j, (band, K, _) in enumerate(sup["subs"]):
                per_super_ops.append((si, [j], band))
    band_cols = [[] for _ in range(NBANDS)]
    col = 0
    for band in range(NBANDS):
        ops_b = [(si, js) for (si, js, b) in per_super_ops if b == band]
        for k, (si, js) in enumerate(ops_b):
            s2ops.append((si, js, band, col + k))
        width = len(ops_b) + (len(ops_b) % 2)
        band_cols[band] = list(range(col, col + width))
        col += width
    return supers, slots, s2ops, band_cols, int(col)


SUPERS, SLOTS, S2OPS, BAND_S2COLS, N_S2COLS = _tables()
N_SLOTS = len(SLOTS)  # 92


def _wtp_pieces():
    """Group weight slots into DMA pieces of equal partition span (p0 + K).
    Returns list of (span, [slot indices])."""
    pieces = []
    cur = []
    cur_k = None
    for j, sl in enumerate(SLOTS):
        K = sl["ch"][1] - sl["ch"][0]
        span = sl["span"]
        Kc = 128 if span > 100 else span
        if K == 15:
            Kc = 15
        if cur_k is None or Kc != cur_k or len(cur) >= 24:
            if cur:
                pieces.append((cur_k, cur))
            cur = []
            cur_k = Kc
        cur.append(j)
    if cur:
        pieces.append((cur_k, cur))
    return pieces


WTP_PIECES = _wtp_pieces()


def _pack_params(W, gamma, beta, bb):
    """Host-side preparation of the parameter-only tensors."""
    Wg = (W * gamma[None, :]).astype(np.float32)
    WgT = np.ascontiguousarray(Wg.T)
    # concatenated class-packed pieces: piece (Kp, js) occupies Kp * len(js) * 128
    total = sum(Kp * len(js) * 128 for (Kp, js) in WTP_PIECES)
    wtp = np.zeros((total,), np.float32)
    off = 0
    for (Kp, js) in WTP_PIECES:
        blk = np.zeros((Kp, len(js), 128), np.float32)
        for k, j in enumerate(js):
            a, e = SLOTS[j]["ch"]
            p0 = SLOTS[j]["p0"]
            blk[p0: p0 + e - a, k, :] = WgT[a:e, :]
        n = blk.size
        wtp[off:off + n] = blk.reshape(-1)
        off += n
    wtp = wtp.reshape(1, -1)
    uvb = np.zeros((128, 2, NBANDS), np.float32)
    for i in range(NBANDS):
        a, e = int(_CHOFF[i]), int(_CHOFF[i + 1])
        uvb[:, 0, i] = Wg[:, a:e].sum(axis=1)
        uvb[:, 1, i] = W[:, a:e] @ beta[a:e] + bb[i]
    return wtp, uvb


def _super_dmas(nc, x_d, sup, xt):
    """Issue the staging DMA(s) for one super tile.

    pair supers need one DMA per c (the (c, f) partition dim is not a single
    stride), cpair supers are a single 3D AP.
    """
    xr = x_d.bitcast(F32R)
    base = xr[0, 0, 0, 0]
    CS = F_TOT * T * 2          # c stride (elements)
    FS = T * 2                  # f stride
    off = sup["f0"] * FS
    if sup["kind"] == "pair64":
        w = sup["w"]
        for j in range(2):  # band half
            for c in range(2):
                p0 = sup["p0"][j] + c * w
                ap = bass.AP(
                    tensor=base.tensor,
                    offset=base.offset + off + j * w * FS + c * CS,
                    ap=[[FS, w], [1, 1024]])
                nc.sync.dma_start(out=xt[p0: p0 + w, j, :], in_=ap)
    elif sup["kind"] == "pair":
        w = sup["w"]
        for c in range(2):
            ap = bass.AP(tensor=base.tensor, offset=base.offset + off + c * CS,
                         ap=[[FS, w], [FS * w, 2], [1, 1024]])
            nc.sync.dma_start(out=xt[c * w: (c + 1) * w, :, :], in_=ap)
    else:
        nf = sup["nf"]
        ap = bass.AP(tensor=base.tensor, offset=base.offset + off,
                     ap=[[FS, nf], [CS, 2], [1, 1024]])
        nc.sync.dma_start(out=xt[0:nf, :, :], in_=ap)


GROUPS = [  # (super index range, band range)
    ((0, 5), (0, 10)),     # class A
    ((5, 11), (10, 22)),   # class B
    ((11, 19), (22, 30)),  # class C
    ((19, 23), (30, 31)),  # class D
]


def _build_nc():
    nc = bacc.Bacc("TRN2")

    x_d = nc.dram_tensor("xb", [C_IN, F_TOT, T, 2], F32, kind="ExternalInput")
    wtp_total = sum(Kp * len(js) * 128 for (Kp, js) in WTP_PIECES)
    wtp_d = nc.dram_tensor("wtp", [1, wtp_total], F32, kind="ExternalInput")
    uvb_d = nc.dram_tensor("uvb", [128, 2, NBANDS], F32, kind="ExternalInput")
    y_d = nc.dram_tensor("y", [OUT_CH, NBANDS, T], F32, kind="ExternalOutput")

    # DRAM scratch for cross-partition broadcasts (written then read back)
    cvec_d = nc.dram_tensor("cvec_scratch", [1, NBANDS], F32)
    rpack_d = nc.dram_tensor("rpack_scratch", [NBANDS, 2], F32)

    # map super index -> group index, and slot traversal order
    sup_group = {}
    for gi, ((s0, s1), _) in enumerate(GROUPS):
        for si in range(s0, s1):
            sup_group[si] = gi

    with tile.TileContext(nc) as tc:
        with tc.tile_pool(name="persist", bufs=1) as persist, \
             tc.tile_pool(name="stage", bufs=12) as stage, \
             tc.tile_pool(name="wtpp", bufs=2) as wtpp, \
             tc.tile_pool(name="osbp", bufs=2) as osbp, \
             tc.tile_pool(name="grp", bufs=2) as grp, \
             tc.tile_pool(name="scratch", bufs=1) as scratchp, \
             tc.tile_pool(name="small", bufs=1) as small, \
             tc.tile_pool(name="psmain", bufs=4, space="PSUM") as psmain, \
             tc.tile_pool(name="pss1", bufs=2, space="PSUM") as pss1, \
             tc.tile_pool(name="pss2", bufs=2, space="PSUM") as pss2:

            # ------------- constants ----------------------------------------
            ohm32 = small.tile([128, 63], F32)
            nc.vector.memset(ohm32, 0.0)
            nc.vector.memset(ohm32[:, 31:32], 1.0)
            ohm = small.tile([128, 63], F32R)
            nc.vector.tensor_copy(out=ohm, in_=ohm32)

            cvec = small.tile([1, NBANDS], F32)
            for i, w in enumerate(WIDTHS):
                nc.vector.memset(cvec[0:1, i:i + 1], 1.0 / (4 * w * T))
            nc.scalar.dma_start(out=cvec_d[:], in_=cvec)

            epst = small.tile([16, 1], F32)
            nc.vector.memset(epst, EPS)

            strip = small.tile([128, N_S2COLS], F32)
            nc.vector.memset(strip, 0.0)

            uvb = persist.tile([128, 2, NBANDS], F32)
            nc.scalar.dma_start(out=uvb, in_=uvb_d[:])

            # wtp piece 0 on the sync queue (gates the very first matmuls),
            # the rest on the act queue.
            wtps = []
            slot_tile = [None] * N_SLOTS
            off = 0
            for p, (Kp, js) in enumerate(WTP_PIECES):
                ns = len(js)
                wt = wtpp.tile([128, ns, 128], F32R, tag="wtpc", name=f"wtp{p}")
                src = wtp_d.bitcast(F32R)[0, off: off + Kp * ns * 128]
                ap = bass.AP(tensor=src.tensor, offset=src.offset,
                             ap=[[ns * 128, Kp], [128, ns], [1, 128]])
                eng = nc.sync if p == 0 else nc.scalar
                eng.dma_start(out=wt[0:Kp, :, :], in_=ap)
                off += Kp * ns * 128
                wtps.append(wt)
                for k, j in enumerate(js):
                    slot_tile[j] = (p, k)

            # ------------- streaming over groups ----------------------------
            band_nmm = {}
            for sl in SLOTS:
                b = SUPERS[sl["super"]]["subs"][sl["sub"]][0]
                band_nmm[b] = band_nmm.get(b, 0) + 1

            slot_iter = 0
            for gi, ((s0, s1), (b0, b1)) in enumerate(GROUPS):
                ng = b1 - b0
                osb = osbp.tile([128, 12, T], F32, tag="osb", name=f"osb{gi}")
                s1g = pss1.tile([32, T], F32, tag="s1g", name=f"s1g{gi}")
                s2g = pss2.tile([32, 24], F32, tag="s2g", name=f"s2g{gi}")
                band_done = {b_: 0 for b_ in range(b0, b1)}
                band_psum = {}
                n_s1 = sum(2 * len(SUPERS[si]["subs"]) for si in range(s0, s1))
                s1_idx = 0
                sup_s2cols = {}

                for si in range(s0, s1):
                    sup = SUPERS[si]
                    nsub = len(sup["subs"])
                    xt = stage.tile([128, nsub, 1024], F32R, tag="xt",
                                    name=f"xt{si}")
                    _super_dmas(nc, x_d, sup, xt)

                    for j, (band, K, chs) in enumerate(sup["subs"]):
                        brel = band - b0
                        p0 = sup.get("p0", [0] * nsub)[j]
                        xv = xt[:, j, :].rearrange("p (t r) -> p t r", r=2)
                        if band not in band_psum:
                            band_psum[band] = psmain.tile(
                                [128, T], F32, tag="acc", name=f"acc{band}")
                        for r in range(2):
                            pi, lj = slot_tile[slot_iter]
                            slot_iter += 1
                            band_done[band] += 1
                            nc.tensor.matmul(
                                band_psum[band][:],
                                wtps[pi][p0:p0 + K, lj, :],
                                xv[p0:p0 + K, :, r],
                                start=(band_done[band] == 1),
                                stop=(band_done[band] == band_nmm[band]),
                            )
                        for h in range(2):
                            s1_idx += 1
                            nc.tensor.matmul(
                                s1g[:],
                                ohm[p0:p0 + K, 31 - brel: 63 - brel],
                                xt[p0:p0 + K, j, h * T: (h + 1) * T],
                                start=(s1_idx == 1),
                                stop=(s1_idx == n_s1),
                            )
                        if band_done[band] == band_nmm[band]:
                            acc = band_psum.pop(band)
                            nc.vector.tensor_copy(out=osb[:, brel, :],
                                                  in_=acc[:])

                    # S2 square+accumulate for this super (scalar engine)
                    for (ssi, js, band, col) in S2OPS:
                        if ssi != si:
                            continue
                        K = sup["subs"][js[0]][1]
                        q0 = sup.get("p0", [0] * nsub)[js[0]]
                        sq = scratchp.tile([128, 2048], F32, tag="sq",
                                           name=f"sq{si}_{js[0]}")
                        if len(js) == 1:
                            in_ap = xt.bitcast(F32)[q0:q0 + K, js[0], :]
                            out_ap = sq[q0:q0 + K, 0:1024]
                        else:
                            in_ap = xt.bitcast(F32)[q0:q0 + K, :, :]
                            out_ap = sq[q0:q0 + K, 0: 1024 * len(js)]
                        nc.scalar.activation(
                            out=out_ap, in_=in_ap, func=AFT.Square,
                            accum_out=strip[q0:q0 + K, col: col + 1],
                        )

                # ---- group statistics ----
                strip_cols = [c for b_ in range(b0, b1) for c in BAND_S2COLS[b_]]
                gc0, gc1 = strip_cols[0], strip_cols[-1] + 1
                strip_r = grp.tile([128, 24], F32R, tag="stripr", name=f"str{gi}")
                nc.vector.tensor_copy(out=strip_r[:, 0: gc1 - gc0],
                                      in_=strip[:, gc0:gc1])
                for k, band in enumerate(range(b0, b1)):
                    cols = BAND_S2COLS[band]
                    c0, c1 = cols[0] - gc0, cols[-1] + 1 - gc0
                    brel = band - b0
                    nc.tensor.matmul(
                        s2g[:, c0:c1],
                        ohm[0:128, 31 - brel: 63 - brel],
                        strip_r[:, c0:c1],
                        start=(k == 0), stop=(k == ng - 1),
                    )

                s1red = grp.tile([16, 1], F32, tag="s1red", name=f"s1r{gi}")
                nc.vector.tensor_reduce(out=s1red[0:ng, :], in_=s1g[0:ng, :],
                                        axis=mybir.AxisListType.X, op=ALU.add)
                s2red = grp.tile([16, 1], F32, tag="s2red", name=f"s2r{gi}")
                nc.vector.tensor_reduce(out=s2red[0:ng, :],
                                        in_=s2g[0:ng, 0: gc1 - gc0],
                                        axis=mybir.AxisListType.X, op=ALU.add)

                invn = grp.tile([16, 1], F32, tag="invn", name=f"inv{gi}")
                src = cvec_d[0:1, b0:b1]
                nc.scalar.dma_start(
                    out=invn[0:ng, :],
                    in_=bass.AP(tensor=src.tensor, offset=src.offset,
                                ap=[[1, ng], [1, 1]]),
                )

                mu = grp.tile([16, 1], F32, tag="mu", name=f"mu{gi}")
                nc.vector.tensor_mul(out=mu[0:ng], in0=s1red[0:ng],
                                     in1=invn[0:ng])
                ex2 = grp.tile([16, 1], F32, tag="ex2", name=f"ex2{gi}")
                nc.vector.tensor_mul(out=ex2[0:ng], in0=s2red[0:ng],
                                     in1=invn[0:ng])
                musq = grp.tile([16, 1], F32, tag="musq", name=f"msq{gi}")
                nc.vector.tensor_mul(out=musq[0:ng], in0=mu[0:ng], in1=mu[0:ng])
                var = grp.tile([16, 1], F32, tag="var", name=f"var{gi}")
                nc.vector.tensor_tensor(out=var[0:ng], in0=ex2[0:ng],
                                        in1=musq[0:ng], op=ALU.subtract)
                std = grp.tile([16, 1], F32, tag="std", name=f"std{gi}")
                nc.scalar.activation(out=std[0:ng], in_=var[0:ng],
                                     func=AFT.Sqrt, bias=epst[0:ng, 0:1])
                rpack = grp.tile([16, 2], F32, tag="rpack", name=f"rp{gi}")
                nc.vector.reciprocal(out=rpack[0:ng, 0:1], in_=std[0:ng])
                nc.vector.tensor_mul(out=rpack[0:ng, 1:2], in0=rpack[0:ng, 0:1],
                                     in1=mu[0:ng])

                nc.scalar.dma_start(out=rpack_d[b0:b1, :], in_=rpack[0:ng, :])
                rbu = grp.tile([128, 12, 2], F32, tag="rbu", name=f"rbu{gi}")
                src_r = rpack_d[b0:b0 + 1, 0:1]
                nc.scalar.dma_start(
                    out=rbu[:, 0:ng, :],
                    in_=bass.AP(tensor=src_r.tensor, offset=src_r.offset,
                                ap=[[0, 128], [2, ng], [1, 2]]),
                )

                t_ru = grp.tile([128, 12], F32, tag="tru", name=f"tru{gi}")
                nc.vector.tensor_mul(out=t_ru[:, 0:ng], in0=rbu[:, 0:ng, 1],
                                     in1=uvb[:, 0, b0:b1])
                bbv = grp.tile([128, 12], F32, tag="bbv", name=f"bbv{gi}")
                nc.vector.tensor_tensor(out=bbv[:, 0:ng], in0=uvb[:, 1, b0:b1],
                                        in1=t_ru[:, 0:ng], op=ALU.subtract)

                # finalize in place (split DVE/ACT) + one grouped store
                for brel in range(ng):
                    if brel % 2 == 0:
                        nc.vector.tensor_scalar(
                            out=osb[:, brel, :], in0=osb[:, brel, :],
                            scalar1=rbu[:, brel, 0:1],
                            scalar2=bbv[:, brel: brel + 1],
                            op0=ALU.mult, op1=ALU.add,
                        )
                    else:
                        nc.scalar.activation(
                            out=osb[:, brel, :], in_=osb[:, brel, :],
                            func=AFT.Identity,
                            scale=rbu[:, brel, 0:1],
                            bias=bbv[:, brel: brel + 1],
                        )
                nc.scalar.dma_start(out=y_d[:, b0:b1, :], in_=osb[:, 0:ng, :])

    nc.finalize()
    return nc


_NC_CACHE = None


def _get_nc():
    global _NC_CACHE
    if _NC_CACHE is None:
        _NC_CACHE = _build_nc()
    return _NC_CACHE


def kernel(x, gamma, beta, W, b):
    from concourse.bass_utils import run_bass_kernel_spmd

    x = np.asarray(x, dtype=np.float32)
    gamma = np.asarray(gamma, dtype=np.float32)
    beta = np.asarray(beta, dtype=np.float32)
    W = np.asarray(W, dtype=np.float32)
    b = np.asarray(b, dtype=np.float32)

    wtp, uvb = _pack_params(W, gamma, beta, b)
    nc = _get_nc()
    in_maps = [
        {"xb": np.ascontiguousarray(x[i]), "wtp": wtp, "uvb": uvb}
        for i in range(N_CORES)
    ]
    res = run_bass_kernel_spmd(nc, in_maps, list(range(N_CORES)))
    return np.stack([res.results[i]["y"] for i in range(N_CORES)], axis=0)

